# revision 23
# baseline (speedup 1.0000x reference)
"""Depthwise 3x3 conv (SAME, channel multiplier 2) on [16,224,224,96] f32,
data-parallel over batch across 8 TRN2 NeuronCores.

Per-core mapping (2 images/core), tap-packed banded matmul on TensorE:
the contract dim packs TWO W-taps of the 3x3 kernel -- K = 122 = 58 x rows
(tap w-1) + 2 bias rows + 4 zero rows + 58 W-shifted duplicate x rows
(tap w) -- and the stationary free dim packs both channel multipliers:
M = 112 = 56 output rows x 2 mult.  Each [112, 512] PSUM chunk then needs
only TWO 512-col matmul streams (stationary A = taps w-1,w + bias;
stationary B = tap w+1) instead of six -> 672 matmuls x 512 cols ~= 143 us
tensor floor, below the ~60 MB/core DMA-engine floor (~200 us: reads run
latency-bound at ~14.7 GB/s/engine, writes ~23.7, 16 engines).

The duplicate half of the moving tile is built on-chip: one DVE copy per
tile shifts the DMA'd top half by +96 cols (one W position) into SBUF
partitions 64:122 (engine APs may start only at partition 0/32/64/96).
Input rides the gpsimd SWDGE queue, whose serial runahead self-prefetches
tiles ~1 ahead of compute; output groups ride the sync HWDGE queue so
evict-completion waits never delay input reads.  The next tile's dup is
emitted mid-tile, off the tile-boundary critical path.  Output leaves
packed [8, 112, 21504] f16 and is unpacked + upcast on the host.
"""

import sys

sys.path.insert(0, "/opt/trn_rl_repo")

import numpy as np

B, H, W, C = 16, 224, 224, 96
MULT = 2
NCORES = 8
BPC = B // NCORES   # images per core
RT = 56             # output rows per h-tile
NHT = H // RT       # 4 h-tiles per image
NTILE = BPC * NHT   # 8 tiles per core
XROWS = 58          # x rows per tile (RT + 2 halo)
KROWS = 64          # DMA partition rows: 58 x + 2 bias + 4 zeros
KP = 122            # contract: 64 + 58 dup rows
XW = (W + 2) * C    # 21696 packed x cols (1-w halo each side)
XWF = XW + C        # 21792 = full DMA width (dup src needs +96)
PADC = 96           # DRAM row pad: keeps partition stride != run so the
                    # DMA descriptor spray uses all 16 engines
XWP = XWF + PADC    # 21888 DRAM pitch
OCOLS = W * C       # 21504 out cols per tile
CHUNK = 512
NCHUNK = OCOLS // CHUNK  # 42
M = 112             # psum partitions: 56 rows x 2 mult

_cache = {}
XDT = "f16"


def _build():
    import concourse.bacc as bacc
    import concourse.tile as tile
    from concourse import mybir

    f32 = mybir.dt.float32
    f16 = mybir.dt.float16

    nc = bacc.Bacc("TRN2", target_bir_lowering=False, debug=False)
    x_d = nc.dram_tensor("x", [NTILE, KROWS, XWP], f16, kind="ExternalInput")
    stat_d = nc.dram_tensor("stat", [KP, 240], f16, kind="ExternalInput")
    out_d = nc.dram_tensor("out", [NTILE, M, OCOLS], f16, kind="ExternalOutput")

    with tile.TileContext(nc) as tc:
        # tile 0 loads as a cascade of independent segment tiles so the PE
        # starts after ~0.5 MB instead of the full 2.8 MB tile.  Segment s
        # covers out chunks [q0, q1): moving cols [q0*512, (q1-1)*512+704).
        SEGQ = (0, 7, 18, 30, 42)
        SEGS = []  # (q0, base_col, end_col)
        for q0, q1 in zip(SEGQ, SEGQ[1:]):
            SEGS.append((q0, q0 * CHUNK, (q1 - 1) * CHUNK + 704))
        with (
            tc.tile_pool(name="const", bufs=1) as const,
            tc.tile_pool(name="xp", bufs=2) as xp,
            tc.tile_pool(name="op", bufs=3) as op,
            tc.tile_pool(name="ps", bufs=6, space="PSUM") as ps,
        ):
            # stationaries go absolutely first -- they gate the first
            # LDWEIGHTS.  statA = cols 0:112, statB = cols 112:224.
            stat_t = const.tile([KP, 224], f16)
            nc.sync.dma_start(stat_t, stat_d[:, 0:224])

            def emit_in(ti):
                if ti >= NTILE:
                    return None
                t = xp.tile([KP, XWF], f16, tag="xt", name=f"xt{ti}")
                # tiles 3+ skip rows 58:64: with 2 rotating buffers those
                # partitions still hold the identical bias/zero rows
                # DMA'd two tiles ago (reads are the scarce resource:
                # ~14.7 GB/s/engine vs ~23.7 for writes)
                hi = KROWS if ti <= 2 else XROWS
                nc.gpsimd.dma_start(t[0:hi, :], x_d[ti][0:hi, 0:XWF])
                return t

            ev = 0  # eviction round-robin DVE/ACT
            xt = None
            for ti in range(NTILE):
                first = ti == 0
                last = ti == NTILE - 1
                if first:
                    segs = []
                    for q0, base, end in SEGS:
                        # DMA 96 extra cols so the dup covers the full
                        # [0, wd) window stream B reads (zero-coef rows
                        # must still be finite -- NaN*0 poisons PSUM)
                        wd = end - base
                        sg = const.tile([KP, wd + 96], f16, tag=f"sg{q0}")
                        nc.gpsimd.dma_start(
                            sg[0:KROWS, :], x_d[ti][:, base : end + 96]
                        )
                        nc.vector.tensor_copy(
                            sg[64:KP, 0:wd], sg[0:XROWS, 96 : wd + 96]
                        )
                        segs.append((q0, base, sg))
                # next tile's input DMA issues now (the gpsimd queue's
                # serial runahead prefetches it during this tile); its
                # dup copy is emitted after group 1 below, so the
                # dup never sits on the tile-boundary critical path
                # (last-evict -> dup -> first-matmul of the next tile)
                nxt = emit_in(ti + 1)

                if last:
                    # shrink the drain tail: progressively smaller groups
                    groups = (12, 10, 8, 6, 4, 2)
                else:
                    groups = (21, 21)
                ch = 0
                for gn, gsz in enumerate(groups):
                    og = op.tile([M, 21 * CHUNK], f16, tag="og")
                    gbase = ch
                    for q in range(gsz):
                        o0 = ch * CHUNK
                        pt = ps.tile([M, CHUNK], f32)
                        if first:
                            mv, cb0 = None, 0
                            for q0, base, sg in segs:
                                if ch >= q0:
                                    mv, cb0 = sg, base
                        else:
                            mv, cb0 = xt, 0
                        nc.tensor.matmul(
                            pt[:, :],
                            stat_t[:, 0:112],
                            mv[0:KP, o0 - cb0 : o0 - cb0 + CHUNK],
                            start=True,
                            stop=False,
                        )
                        nc.tensor.matmul(
                            pt[:, :],
                            stat_t[:, 112:224],
                            mv[0:KP, o0 - cb0 + 192 : o0 - cb0 + 704],
                            start=False,
                            stop=True,
                        )
                        dst = og[:, (ch - gbase) * CHUNK : (ch - gbase + 1) * CHUNK]
                        # DVE also carries the dup copies; weight the
                        # round-robin toward the scalar engine
                        if ev % 8 < 3:
                            nc.vector.tensor_copy(dst, pt[:, :])
                        else:
                            nc.scalar.copy(dst, pt[:, :])
                        ev += 1
                        ch += 1
                    # output DMA triggers live on the sync ring so their
                    # copy-completion waits never block the copy engines
                    nc.sync.dma_start(
                        out_d[ti][:, gbase * CHUNK : ch * CHUNK],
                        og[:, 0 : gsz * CHUNK],
                    )
                    if gn == 0 and nxt is not None:
                        # next tile's dup: its input DMA completed during
                        # this tile's first group, and DVE has slack here
                        nc.vector.tensor_copy(
                            nxt[64:KP, 0:XW], nxt[0:XROWS, 96:XWF]
                        )
                xt = nxt
    nc.compile()
    return nc


def _host_stat(kern, bias):
    kk = np.asarray(kern, np.float32).reshape(3, 3, MULT)  # [i, jw, m]
    statA = np.zeros((KP, M), np.float32)
    statB = np.zeros((KP, M), np.float32)
    js = np.arange(RT)
    for m in range(MULT):
        for i in range(3):
            statA[js + i, m * RT + js] = kk[i, 0, m]
            statA[64 + js + i, m * RT + js] = kk[i, 1, m]
            statB[js + i, m * RT + js] = kk[i, 2, m]
        statA[58 + m, m * RT : (m + 1) * RT] = 1.0
    stat = np.zeros((KP, 240), np.float16)
    stat[:, 0:M] = statA
    stat[:, M : 2 * M] = statB
    return stat


def _pack_inputs(x, kern, bias):
    """Full f32 x [16,224,224,96] -> per-core packed f16 tiles + stationary."""
    stat = _host_stat(kern, bias)
    bias = np.asarray(bias, np.float32)
    # bias row m: value at moving col t = bias[2*(t%96) + m]
    brows = np.empty((MULT, XWP), np.float16)
    for m in range(MULT):
        brows[m] = np.tile(bias[m::MULT].astype(np.float16), XWP // C)
    x16 = np.asarray(x).astype(np.float16)
    in_maps = []
    for core in range(NCORES):
        xc = x16[core * BPC : (core + 1) * BPC]  # [2, 224, 224, 96]
        # xrow[r] = image row r-1 (rows 0 and 225 are the SAME-pad zeros);
        # cols 96:21600 hold w=0..223, zeros elsewhere
        xrow = np.zeros((BPC, H + 2, XWP), np.float16)
        xrow[:, 1 : H + 1, C : C + OCOLS] = xc.reshape(BPC, H, OCOLS)
        xa = np.zeros((NTILE, KROWS, XWP), np.float16)
        for b in range(BPC):
            for ht in range(NHT):
                t = b * NHT + ht
                xa[t, 0:XROWS] = xrow[b, ht * RT : ht * RT + XROWS]
                xa[t, XROWS : XROWS + MULT] = brows
        in_maps.append({"x": xa, "stat": stat})
    return in_maps


def _unpack_output(res):
    """Per-core packed [NTILE, M, OCOLS] f16 -> full [16,224,224,192] f32."""
    outs = []
    for core in range(NCORES):
        oc = np.asarray(res.results[core]["out"])
        # [b, ht, m, j, w, c] -> [b, ht*RT+j, w, c*2+m]
        oc = oc.reshape(BPC, NHT, MULT, RT, W, C)
        oc = oc.transpose(0, 1, 3, 4, 5, 2).reshape(BPC, H, W, C * MULT)
        outs.append(oc.astype(np.float32))
    return np.concatenate(outs, axis=0)


def kernel(**inputs):
    in_maps = _pack_inputs(inputs["x"], inputs["kernel"], inputs["bias"])

    if "nc" not in _cache:
        _cache["nc"] = _build()
    nc = _cache["nc"]

    from concourse.bass_utils import run_bass_kernel_spmd

    res = run_bass_kernel_spmd(nc, in_maps, list(range(NCORES)))
    return _unpack_output(res)
